# revision 24
# baseline (speedup 1.0000x reference)
"""Trainium2 Bass kernel for per-token head-attention transformer block.

Reference computation (N=16, T=4096, D=1024, H=16, hd=64):
    qkv = x @ w_qkv + b_qkv                       (N,T,3D)
    q,k,v = split(qkv)  each (N,T,H,hd)
    S = einsum('nthd,ntgd->nthg', q*hd^-0.5, k)   per-token 16x16 over heads
    P = softmax(S, -1)
    o = einsum('nthg,ntgd->nthd', P, v)
    out = o.transpose(0,2,1,3).reshape(N,T,D) @ w_proj + b_proj

Mapping: data-parallel over batch N across 8 cores (2 batch elements each).
Per core the kernel is vector-engine-bound (the per-token 16x16 attention is
~32K elementwise ops/token/phase); tuning knobs:
  - QK_GROUPS / AV_GROUPS: (h0, nh, engine) head-group split between DVE
    ('v', bf16 tensor_tensor at 2x mode) and GPSIMD ('p'); balance by
    max(DVE_time, pool_penalty * Pool_time) where pool_penalty is 1.0 per
    the CoreSim cost model but ~2.5 per measured Q7 two-input throughput.
  - pool bufs: xp/xtp depth pipelines the phase-2 (projection) chunks;
    psA/psT split the 8 PSUM banks between matmul accumulation and
    transpose-pair evacuation.
Design points:
  - all matmuls in bf16 (same PE rate as f32r, half the SBUF, FWL weight loads)
  - attention math restructured: one fused product per head-group, then an
    in-place halving add-tree (writes back into the product tile's lower
    half); the last level writes S / O directly
  - no max-subtraction in softmax (scores are O(5) for this data; exp in f32)
  - a tunable slice of the head-groups runs on GPSIMD (tensor_tensor never
    contends with DVE ports), overlapping the two vector engines
  - v is written straight into its (d,g)-interleaved tile during the
    PSUM-evacuation of the v transposes (no separate re-copy, no v section
    in the token-major buffer)
  - PSUM evacuations all on ACT; DVE does only attention math
  - attention output spilled to DRAM in bf16 (halves spill traffic)
"""

import sys

sys.path.insert(0, "/opt/trn_rl_repo")

from contextlib import ExitStack

import numpy as np

import concourse.bass as bass
import concourse.tile as tile
from concourse import mybir
from concourse.bass_utils import run_bass_kernel_spmd
from concourse.masks import make_identity

N, T, D = 16, 4096, 1024
H, HD = 16, 64
NCORES = 8
NB = N // NCORES  # batch elements per core
SCALE = float(HD) ** -0.5

F32 = mybir.dt.float32
BF16 = mybir.dt.bfloat16
ATT_DT = BF16

CH = 256          # token chunk (matmul moving dim)
NT = CH // 128    # token tiles per chunk
KD = D // 128     # contraction chunks (8)
JQ = 3 * D // 128  # qkv output feature chunks (24)
JP = D // 128     # proj output feature chunks (8)
NCH = T // CH     # chunks per batch element (16)

# head-group split across the two vector engines: list of (h0, nh, engine)
# engine: 'v' = DVE, 'p' = GPSIMD
# GPSIMD (Pool) shares its SBUF port with DVE: HW-measured, independent Pool
# tensor_tensor ops overlap only ~15% with a busy DVE stream, so offloading
# head-groups to Pool is a net loss. All attention math runs on DVE.
QK_GROUPS = [(0, 16, "v")]
AV_GROUPS = [(0, 16, "v")]

Ident = mybir.ActivationFunctionType.Identity
Exp = mybir.ActivationFunctionType.Exp
ALU = mybir.AluOpType
AX = mybir.AxisListType


def _ap(sl, dims):
    """Custom free-dim access pattern on a sliced tile: keep partition dim +
    offset of `sl`, replace free dims with [step, num] list `dims`."""
    return bass.AP(tensor=sl.tensor, offset=sl.offset, ap=[sl.ap[0]] + dims)


def build_kernel():
    nc = bass.Bass()
    x = nc.dram_tensor("x", [NB * T, D], F32, kind="ExternalInput")
    wqkv = nc.dram_tensor("w_qkv", [D, 3 * D], F32, kind="ExternalInput")
    bqkv = nc.dram_tensor("b_qkv", [3 * D], F32, kind="ExternalInput")
    wproj = nc.dram_tensor("w_proj", [D, D], F32, kind="ExternalInput")
    bproj = nc.dram_tensor("b_proj", [D], F32, kind="ExternalInput")
    y = nc.dram_tensor("y", [NB * T, D], F32, kind="ExternalOutput")

    with ExitStack() as ctx:
        tc = ctx.enter_context(tile.TileContext(nc))
        singles = ctx.enter_context(tc.tile_pool(name="singles", bufs=1))
        xp = ctx.enter_context(tc.tile_pool(name="xp", bufs=2))
        xtp = ctx.enter_context(tc.tile_pool(name="xtp", bufs=3))
        ytp = ctx.enter_context(tc.tile_pool(name="ytp", bufs=1))
        qkvp = ctx.enter_context(tc.tile_pool(name="qkvp", bufs=1))
        tokp = ctx.enter_context(tc.tile_pool(name="tokp", bufs=2))
        att = ctx.enter_context(tc.tile_pool(name="att", bufs=2))
        outp = ctx.enter_context(tc.tile_pool(name="outp", bufs=2))
        prodp = ctx.enter_context(tc.tile_pool(name="prodp", bufs=1))
        vtp = ctx.enter_context(tc.tile_pool(name="vtp", bufs=2))
        psA = ctx.enter_context(tc.tile_pool(name="psA", bufs=3, space="PSUM"))
        psT = ctx.enter_context(tc.tile_pool(name="psT", bufs=5, space="PSUM"))
        dram = ctx.enter_context(tc.tile_pool(name="dram", bufs=1, space="DRAM"))

        ident = singles.tile([128, 128], F32)
        make_identity(nc, ident)
        ident_b = singles.tile([128, 128], ATT_DT)
        make_identity(nc, ident_b)

        # resident weights in bf16, (in,out) layout chunked over contraction dim
        wq_s = singles.tile([128, KD, 3 * D], BF16)
        wq_src = wqkv.rearrange("(k p) j -> p k j", p=128)
        for k in range(KD):
            nc.gpsimd.dma_start(out=wq_s[:, k, :], in_=wq_src[:, k, :])
        wp_s = singles.tile([128, KD, D], BF16)
        wp_src = wproj.rearrange("(k p) j -> p k j", p=128)
        for k in range(KD):
            nc.gpsimd.dma_start(out=wp_s[:, k, :], in_=wp_src[:, k, :])
        # biases, one merged tile: cols [0,JQ) = b_qkv, [JQ,JQ+JP) = b_proj,
        # [JQ+JP, ...) = SCALE * b_q
        bias = singles.tile([128, JQ + JP + JQ // 3], F32)
        nc.gpsimd.dma_start(
            out=bias[:, 0:JQ], in_=bqkv.rearrange("(j p) -> p j", p=128)
        )
        nc.gpsimd.dma_start(
            out=bias[:, JQ : JQ + JP], in_=bproj.rearrange("(j p) -> p j", p=128)
        )
        nc.scalar.mul(bias[:, JQ + JP :], bias[:, 0 : JQ // 3], SCALE)

        # head-major attention-output spill (bf16): flat layout
        # h*(T*HD) + t*HD + d, viewed by phase 2 as a row-major (T, D)
        # matrix per batch element.
        aspill = dram.tile([NB, T, D], ATT_DT)

        def halving_tree(prod, nh, mid, inner, out_final):
            """Sum the innermost dim of prod [128, nh, mid, inner] by
            repeated in-place halving adds (out aliases the low half of
            the input range); the final level writes out_final."""
            m = inner
            while m > 1:
                half = m // 2
                if half == 1:
                    dst = out_final
                else:
                    dst = prod[:, :, :, 0:half]
                nc.vector.tensor_tensor(
                    out=dst,
                    in0=prod[:, :, :, 0:half],
                    in1=prod[:, :, :, half:m],
                    op=ALU.add,
                )
                m = half

        def p1_chunk(n, c):
            t0 = c * CH
            xT = xtp.tile([128, KD, CH], BF16, tag="xT")
            for tt in range(NT):
                xt = xp.tile([128, D], F32, tag="x")
                r0 = n * T + t0 + tt * 128
                nc.sync.dma_start(out=xt, in_=x[r0 : r0 + 128, :])
                for k in range(0, KD, 2):
                    pt = psT.tile([128, 2, 128], F32, tag="tp")
                    for kk in range(2):
                        nc.tensor.transpose(
                            pt[:, kk, :],
                            xt[:, (k + kk) * 128 : (k + kk + 1) * 128],
                            ident,
                        )
                    nc.scalar.copy(
                        out=_ap(xT[:, k, tt * 128], [[CH, 2], [1, 128]]),
                        in_=pt,
                    )

            qkvT = qkvp.tile([128, JQ, CH], ATT_DT, tag="qkvT")
            for j in range(JQ):
                pm = psA.tile([128, CH], F32, tag="mm")
                for k in range(KD):
                    nc.tensor.matmul(
                        pm,
                        wq_s[:, k, j * 128 : (j + 1) * 128],
                        xT[:, k, :],
                        start=(k == 0),
                        stop=(k == KD - 1),
                    )
                if j < JQ // 3:  # q: fold in attention scale
                    nc.scalar.activation(
                        out=qkvT[:, j, :], in_=pm, func=Ident,
                        bias=bias[:, JQ + JP + j : JQ + JP + j + 1], scale=SCALE,
                    )
                else:
                    nc.scalar.activation(
                        out=qkvT[:, j, :], in_=pm, func=Ident,
                        bias=bias[:, j : j + 1], scale=1.0,
                    )

            # token-major marshalling for BOTH tiles first, so the ACT queue
            # never has an exp (which waits on DVE) ahead of evac work
            toks = []
            for tt in range(NT):
                # feature-major -> token-major for the per-token attention;
                # q,k go to `tok`, v goes straight into the (d,g) layout
                tok = tokp.tile([128, 2 * D], ATT_DT, tag="tok")
                vt = vtp.tile([128, HD, H], ATT_DT, tag="vt")
                for j in range(0, JQ, 2):
                    pt = psT.tile([128, 2, 128], ATT_DT, tag="tp")
                    for jj in range(2):
                        nc.tensor.transpose(
                            pt[:, jj, :],
                            qkvT[:, j + jj, tt * 128 : (tt + 1) * 128],
                            ident_b,
                        )
                    if j < 2 * JQ // 3:
                        nc.scalar.copy(
                            out=tok[:, j * 128 : (j + 2) * 128], in_=pt
                        )
                    else:
                        jv = j - 2 * JQ // 3
                        nc.scalar.copy(
                            out=_ap(
                                vt[:, 0, 2 * jv], [[2, 2], [1, 2], [H, HD]]
                            ),
                            in_=_ap(
                                pt[:, 0, 0], [[128, 2], [HD, 2], [1, HD]]
                            ),
                        )
                toks.append((tok, vt))

            # QK for both tiles back-to-back on DVE; the tile-tt exp (ACT)
            # then lands during tile-(tt+1)'s QK, so the later softmax ops
            # never stall the DVE stream
            Ss = []
            for tt in range(NT):
                # scores S[t,h,g] = sum_d q[t,h,d] k[t,g,d] (q pre-scaled)
                tok = toks[tt][0]
                S = att.tile([128, H, H], F32, tag="S")
                ks = tok[:, D : 2 * D]
                prod = prodp.tile([128, H, H, HD], ATT_DT, tag="prod")
                qs = tok[:, 0 : H * HD]
                nc.vector.tensor_tensor(
                    out=prod,
                    in0=_ap(qs, [[HD, H], [0, H], [1, HD]]),
                    in1=_ap(ks, [[0, H], [HD, H], [1, HD]]),
                    op=ALU.mult,
                )
                sdst = _ap(S[:, 0, 0], [[H, H], [1, H], [1, 1]])
                halving_tree(prod, H, H, HD, sdst)
                Ss.append(S)

            for tt in range(NT):
                vt = toks[tt][1]
                # softmax over g without max-subtraction (|S| is O(5));
                # all side-ops on DVE (Pool contends with DVE for SBUF ports)
                Sx = att.tile([128, H, H], ATT_DT, tag="Sx")
                nc.scalar.activation(out=Sx, in_=Ss[tt], func=Exp)
                stats = att.tile([128, 2, H], F32, tag="stats")
                nc.vector.tensor_reduce(
                    out=stats[:, 0, :], in_=Sx, axis=AX.X, op=ALU.add
                )
                nc.vector.reciprocal(stats[:, 1, :], stats[:, 0, :])
                P = att.tile([128, H, H], ATT_DT, tag="P")
                nc.vector.tensor_tensor(
                    out=P, in0=Sx, in1=_ap(stats[:, 1, 0], [[1, H], [0, H]]),
                    op=ALU.mult,
                )

                # o[t,h,d] = sum_g P[t,h,g] v[t,g,d]
                O = outp.tile([128, H, HD], ATT_DT, tag="O")
                prod2 = prodp.tile([128, H, HD, H], ATT_DT, tag="prod")
                nc.vector.tensor_tensor(
                    out=prod2,
                    in0=_ap(P[:, 0, :], [[H, H], [0, HD], [1, H]]),
                    in1=_ap(vt[:, :], [[0, H], [H, HD], [1, H]]),
                    op=ALU.mult,
                )
                odst = _ap(O[:, 0, :], [[HD, H], [1, HD], [1, 1]])
                halving_tree(prod2, H, HD, H, odst)

                # spill head-major: dst[h, t, d] = O[t, h, d]; one DMA per
                # head on the idle Pool queue so each write's DRAM range is
                # exact and p2 halves can start as soon as their token-half
                # has spilled (Tile DMA dep tracking is [min,max) ranges)
                base = aspill[n]
                for h in range(H):
                    dst = bass.AP(
                        tensor=base.tensor,
                        offset=base.offset + h * T * HD + (t0 + tt * 128) * HD,
                        ap=[[HD, 128], [1, HD]],
                    )
                    nc.gpsimd.dma_start(out=dst, in_=O[:, h, :])

        def p2_half(n, c, half, tail=False):
            # one 128-row block of the (T, D) view = head c, token half
            # `half`; in the tail (no attention running) DVE is idle: split
            # the PSUM evacuations between DVE and ACT and keep the
            # spill-load off the ACT queue so ACT never blocks PE
            evac = 0

            def tail_cp(**kw):
                nonlocal evac
                evac += 1
                if evac % 2:
                    nc.vector.tensor_copy(**kw)
                else:
                    nc.scalar.copy(**kw)

            r0v = c * CH + half * 128
            at = xp.tile([128, D], ATT_DT, tag="a")
            dma = nc.sync.dma_start if tail else nc.scalar.dma_start
            dma(out=at, in_=aspill[n, r0v : r0v + 128, :])
            ATh = xtp.tile([128, KD, 128], ATT_DT, tag="AT")
            for k in range(0, KD, 2):
                pt = psT.tile([128, 2, 128], ATT_DT, tag="tp")
                for kk in range(2):
                    nc.tensor.transpose(
                        pt[:, kk, :],
                        at[:, (k + kk) * 128 : (k + kk + 1) * 128],
                        ident_b,
                    )
                cp = tail_cp if tail else nc.scalar.copy
                cp(out=_ap(ATh[:, k, 0], [[128, 2], [1, 128]]), in_=pt)
            yTh = ytp.tile([128, JP, 128], ATT_DT, tag="yT")
            for j in range(JP):
                pm = psA.tile([128, 128], F32, tag="mm")
                for k in range(KD):
                    nc.tensor.matmul(
                        pm,
                        wp_s[:, k, j * 128 : (j + 1) * 128],
                        ATh[:, k, :],
                        start=(k == 0),
                        stop=(k == KD - 1),
                    )
                nc.scalar.activation(
                    out=yTh[:, j, :], in_=pm, func=Ident,
                    bias=bias[:, JQ + j : JQ + j + 1], scale=1.0,
                )
            yt = ytp.tile([128, D], F32, tag="y")
            for j in range(0, JP, 2):
                pt = psT.tile([128, 2, 128], ATT_DT, tag="tp")
                for jj in range(2):
                    nc.tensor.transpose(
                        pt[:, jj, :], yTh[:, j + jj, :], ident_b
                    )
                cp = tail_cp if tail else nc.scalar.copy
                cp(out=yt[:, j * 128 : (j + 2) * 128], in_=pt)
            r0 = n * T + r0v
            nc.sync.dma_start(out=y[r0 : r0 + 128, :], in_=yt)

        # interleave phase-2 halves behind phase-1: a (head c, half) block
        # of the projection needs only the spills of p1(n, half*8..half*8+7)
        # thanks to the per-head spill DMAs, so p2 halves drain at 2 per p1
        # chunk and only the last batch element's half-1 runs as a tail
        pending = []

        def drain(k, tail=False):
            for _ in range(min(k, len(pending))):
                m, cc, hh = pending.pop(0)
                p2_half(m, cc, hh, tail)

        for n in range(NB):
            for c in range(NCH):
                p1_chunk(n, c)
                drain(2)
                if c == NCH // 2 - 1:
                    pending += [(n, cc, 0) for cc in range(NCH)]
                elif c == NCH - 1:
                    pending += [(n, cc, 1) for cc in range(NCH)]
        while pending:
            drain(4, tail=True)

    # TRN2 allows at most one sync wait per engine instruction; split
    # multi-wait instructions through event semaphores.
    import bass_rust

    bass_rust.generate_event_semaphores(nc)
    return nc


_NC_CACHE = None
TRACE = False
LAST_RESULTS = None


def kernel(x, w_qkv, b_qkv, w_proj, b_proj):
    global _NC_CACHE, LAST_RESULTS
    if _NC_CACHE is None:
        _NC_CACHE = build_kernel()
    nc = _NC_CACHE
    x = np.ascontiguousarray(np.asarray(x, dtype=np.float32))
    w_qkv = np.ascontiguousarray(np.asarray(w_qkv, dtype=np.float32))
    b_qkv = np.ascontiguousarray(np.asarray(b_qkv, dtype=np.float32))
    w_proj = np.ascontiguousarray(np.asarray(w_proj, dtype=np.float32))
    b_proj = np.ascontiguousarray(np.asarray(b_proj, dtype=np.float32))
    in_maps = []
    for i in range(NCORES):
        in_maps.append(
            {
                "x": x[i * NB : (i + 1) * NB].reshape(NB * T, D),
                "w_qkv": w_qkv,
                "b_qkv": b_qkv,
                "w_proj": w_proj,
                "b_proj": b_proj,
            }
        )
    res = run_bass_kernel_spmd(
        nc, in_maps, core_ids=list(range(NCORES)), trace=TRACE
    )
    LAST_RESULTS = res
    out = np.empty((N, T, D), dtype=np.float32)
    for i in range(NCORES):
        out[i * NB : (i + 1) * NB] = res.results[i]["y"].reshape(NB, T, D)
    return out


if __name__ == "__main__":
    rng = np.random.default_rng(0)
    inputs = {
        "x": rng.standard_normal((N, T, D), dtype=np.float32),
        "w_qkv": rng.standard_normal((D, 3 * D), dtype=np.float32) * D**-0.5,
        "b_qkv": rng.standard_normal((3 * D,), dtype=np.float32) * 0.02,
        "w_proj": rng.standard_normal((D, D), dtype=np.float32) * D**-0.5,
        "b_proj": rng.standard_normal((D,), dtype=np.float32) * 0.02,
    }
    out = kernel(**inputs)
    print("out", out.shape, out.dtype)



# revision 27
# speedup vs baseline: 2.2846x; 2.2846x over previous
"""Trainium2 Bass kernel for per-token head-attention transformer block.

Reference computation (N=16, T=4096, D=1024, H=16, hd=64):
    qkv = x @ w_qkv + b_qkv                       (N,T,3D)
    q,k,v = split(qkv)  each (N,T,H,hd)
    S = einsum('nthd,ntgd->nthg', q*hd^-0.5, k)   per-token 16x16 over heads
    P = softmax(S, -1)
    o = einsum('nthg,ntgd->nthd', P, v)
    out = o.transpose(0,2,1,3).reshape(N,T,D) @ w_proj + b_proj

Mapping: data-parallel over batch N across 8 cores (2 batch elements each).
Per core the kernel is DVE-bound: the per-token 16x16 attention costs ~32K
MACs/token which runs at ~1 MAC/cycle/lane as bf16 tensor_tensor product +
halving add-tree (~40us per 128-token tile on HW incl. ~0.4us/op overhead).
Design points (HW-measured rationale in the session notes):
  - ALL attention math on DVE. GPSIMD shares its SBUF port with DVE;
    measured: independent Pool tensor_tensor overlaps only ~15% with a busy
    DVE stream, so any head-group offload to Pool is a net loss.
  - per-chunk emission order: tok/v marshalling for BOTH tiles, then QK for
    both tiles back-to-back on DVE, then per-tile softmax+AV. The tile-tt
    exp (ACT) lands during tile-(tt+1)'s QK, so softmax never stalls DVE,
    and ACT's queue never has an exp ahead of evacuation work
    (head-of-line blocking starves PE via psT/psA backpressure otherwise).
  - all matmuls in bf16 (same PE rate as f32r, half the SBUF, FWL weight loads)
  - one fused product per tile+phase, then an in-place halving add-tree;
    the last level writes S / O directly
  - no max-subtraction in softmax (scores are O(5) for this data; exp in f32)
  - v is written straight into its (d,g)-interleaved tile during the
    PSUM-evacuation of the v transposes
  - PSUM evacuations all on ACT; DVE does only attention math + softmax
    side-ops (reduce/recip/normalize are cheaper on DVE than the cross-engine
    stall they cause elsewhere)
  - attention output spilled to DRAM in bf16; p2(n, c) projects head-c rows
    of the reference's h-major (T,D) view, which span ALL tokens of batch
    element n, so p2 lags p1 by one full batch element
Measured on HW: baseline 4.19ms -> 2.92ms. Failed experiments (reverted):
head-splits to Pool (port contention), tile-merged tree ops (exp head-of-line
cascade, 3.44ms), per-head spill DMAs + half-granular p2 (SWDGE DMA storm,
7.6ms), reciprocal_approx_fast (neuronxcc "ISA wrong length").
"""

import sys

sys.path.insert(0, "/opt/trn_rl_repo")

from contextlib import ExitStack

import numpy as np

import concourse.bass as bass
import concourse.tile as tile
from concourse import mybir
from concourse.bass_utils import run_bass_kernel_spmd
from concourse.masks import make_identity

N, T, D = 16, 4096, 1024
H, HD = 16, 64
NCORES = 8
NB = N // NCORES  # batch elements per core
SCALE = float(HD) ** -0.5

F32 = mybir.dt.float32
BF16 = mybir.dt.bfloat16
ATT_DT = BF16

CH = 256          # token chunk (matmul moving dim)
NT = CH // 128    # token tiles per chunk
KD = D // 128     # contraction chunks (8)
JQ = 3 * D // 128  # qkv output feature chunks (24)
JP = D // 128     # proj output feature chunks (8)
NCH = T // CH     # chunks per batch element (16)

# head-group split across the two vector engines: list of (h0, nh, engine)
# engine: 'v' = DVE, 'p' = GPSIMD
# GPSIMD (Pool) shares its SBUF port with DVE: HW-measured, independent Pool
# tensor_tensor ops overlap only ~15% with a busy DVE stream, so offloading
# head-groups to Pool is a net loss. All attention math runs on DVE.
QK_GROUPS = [(0, 16, "v")]
AV_GROUPS = [(0, 16, "v")]

Ident = mybir.ActivationFunctionType.Identity
Exp = mybir.ActivationFunctionType.Exp
ALU = mybir.AluOpType
AX = mybir.AxisListType


def _ap(sl, dims):
    """Custom free-dim access pattern on a sliced tile: keep partition dim +
    offset of `sl`, replace free dims with [step, num] list `dims`."""
    return bass.AP(tensor=sl.tensor, offset=sl.offset, ap=[sl.ap[0]] + dims)


def build_kernel():
    nc = bass.Bass()
    x = nc.dram_tensor("x", [NB * T, D], F32, kind="ExternalInput")
    wqkv = nc.dram_tensor("w_qkv", [D, 3 * D], F32, kind="ExternalInput")
    bqkv = nc.dram_tensor("b_qkv", [3 * D], F32, kind="ExternalInput")
    wproj = nc.dram_tensor("w_proj", [D, D], F32, kind="ExternalInput")
    bproj = nc.dram_tensor("b_proj", [D], F32, kind="ExternalInput")
    y = nc.dram_tensor("y", [NB * T, D], F32, kind="ExternalOutput")

    with ExitStack() as ctx:
        tc = ctx.enter_context(tile.TileContext(nc))
        singles = ctx.enter_context(tc.tile_pool(name="singles", bufs=1))
        xp = ctx.enter_context(tc.tile_pool(name="xp", bufs=2))
        xtp = ctx.enter_context(tc.tile_pool(name="xtp", bufs=3))
        ytp = ctx.enter_context(tc.tile_pool(name="ytp", bufs=1))
        qkvp = ctx.enter_context(tc.tile_pool(name="qkvp", bufs=1))
        tokp = ctx.enter_context(tc.tile_pool(name="tokp", bufs=2))
        att = ctx.enter_context(tc.tile_pool(name="att", bufs=2))
        outp = ctx.enter_context(tc.tile_pool(name="outp", bufs=2))
        prodp = ctx.enter_context(tc.tile_pool(name="prodp", bufs=1))
        vtp = ctx.enter_context(tc.tile_pool(name="vtp", bufs=2))
        psA = ctx.enter_context(tc.tile_pool(name="psA", bufs=3, space="PSUM"))
        psT = ctx.enter_context(tc.tile_pool(name="psT", bufs=5, space="PSUM"))
        dram = ctx.enter_context(tc.tile_pool(name="dram", bufs=1, space="DRAM"))

        ident = singles.tile([128, 128], F32)
        make_identity(nc, ident)
        ident_b = singles.tile([128, 128], ATT_DT)
        make_identity(nc, ident_b)

        # resident weights in bf16, (in,out) layout chunked over contraction dim
        wq_s = singles.tile([128, KD, 3 * D], BF16)
        wq_src = wqkv.rearrange("(k p) j -> p k j", p=128)
        for k in range(KD):
            nc.gpsimd.dma_start(out=wq_s[:, k, :], in_=wq_src[:, k, :])
        wp_s = singles.tile([128, KD, D], BF16)
        wp_src = wproj.rearrange("(k p) j -> p k j", p=128)
        for k in range(KD):
            nc.gpsimd.dma_start(out=wp_s[:, k, :], in_=wp_src[:, k, :])
        # biases, one merged tile: cols [0,JQ) = b_qkv, [JQ,JQ+JP) = b_proj,
        # [JQ+JP, ...) = SCALE * b_q
        bias = singles.tile([128, JQ + JP + JQ // 3], F32)
        nc.gpsimd.dma_start(
            out=bias[:, 0:JQ], in_=bqkv.rearrange("(j p) -> p j", p=128)
        )
        nc.gpsimd.dma_start(
            out=bias[:, JQ : JQ + JP], in_=bproj.rearrange("(j p) -> p j", p=128)
        )
        nc.scalar.mul(bias[:, JQ + JP :], bias[:, 0 : JQ // 3], SCALE)

        # head-major attention-output spill (bf16): flat layout
        # h*(T*HD) + t*HD + d, viewed by phase 2 as a row-major (T, D)
        # matrix per batch element.
        aspill = dram.tile([NB, T, D], ATT_DT)

        def halving_tree(prod, nh, mid, inner, out_final):
            """Sum the innermost dim of prod [128, nh, mid, inner] by
            repeated in-place halving adds (out aliases the low half of
            the input range); the final level writes out_final."""
            m = inner
            while m > 1:
                half = m // 2
                if half == 1:
                    dst = out_final
                else:
                    dst = prod[:, :, :, 0:half]
                nc.vector.tensor_tensor(
                    out=dst,
                    in0=prod[:, :, :, 0:half],
                    in1=prod[:, :, :, half:m],
                    op=ALU.add,
                )
                m = half

        def p1_chunk(n, c):
            t0 = c * CH
            xT = xtp.tile([128, KD, CH], BF16, tag="xT")
            for tt in range(NT):
                xt = xp.tile([128, D], F32, tag="x")
                r0 = n * T + t0 + tt * 128
                nc.sync.dma_start(out=xt, in_=x[r0 : r0 + 128, :])
                for k in range(0, KD, 2):
                    pt = psT.tile([128, 2, 128], F32, tag="tp")
                    for kk in range(2):
                        nc.tensor.transpose(
                            pt[:, kk, :],
                            xt[:, (k + kk) * 128 : (k + kk + 1) * 128],
                            ident,
                        )
                    nc.scalar.copy(
                        out=_ap(xT[:, k, tt * 128], [[CH, 2], [1, 128]]),
                        in_=pt,
                    )

            qkvT = qkvp.tile([128, JQ, CH], ATT_DT, tag="qkvT")
            for j in range(JQ):
                pm = psA.tile([128, CH], F32, tag="mm")
                for k in range(KD):
                    nc.tensor.matmul(
                        pm,
                        wq_s[:, k, j * 128 : (j + 1) * 128],
                        xT[:, k, :],
                        start=(k == 0),
                        stop=(k == KD - 1),
                    )
                if j < JQ // 3:  # q: fold in attention scale
                    nc.scalar.activation(
                        out=qkvT[:, j, :], in_=pm, func=Ident,
                        bias=bias[:, JQ + JP + j : JQ + JP + j + 1], scale=SCALE,
                    )
                else:
                    nc.scalar.activation(
                        out=qkvT[:, j, :], in_=pm, func=Ident,
                        bias=bias[:, j : j + 1], scale=1.0,
                    )

            # token-major marshalling for BOTH tiles first, so the ACT queue
            # never has an exp (which waits on DVE) ahead of evac work
            toks = []
            for tt in range(NT):
                # feature-major -> token-major for the per-token attention;
                # q,k go to `tok`, v goes straight into the (d,g) layout
                tok = tokp.tile([128, 2 * D], ATT_DT, tag="tok")
                vt = vtp.tile([128, HD, H], ATT_DT, tag="vt")
                for j in range(0, JQ, 2):
                    pt = psT.tile([128, 2, 128], ATT_DT, tag="tp")
                    for jj in range(2):
                        nc.tensor.transpose(
                            pt[:, jj, :],
                            qkvT[:, j + jj, tt * 128 : (tt + 1) * 128],
                            ident_b,
                        )
                    if j < 2 * JQ // 3:
                        nc.scalar.copy(
                            out=tok[:, j * 128 : (j + 2) * 128], in_=pt
                        )
                    else:
                        jv = j - 2 * JQ // 3
                        nc.scalar.copy(
                            out=_ap(
                                vt[:, 0, 2 * jv], [[2, 2], [1, 2], [H, HD]]
                            ),
                            in_=_ap(
                                pt[:, 0, 0], [[128, 2], [HD, 2], [1, HD]]
                            ),
                        )
                toks.append((tok, vt))

            # QK for both tiles back-to-back on DVE; the tile-tt exp (ACT)
            # then lands during tile-(tt+1)'s QK, so the later softmax ops
            # never stall the DVE stream
            Ss = []
            for tt in range(NT):
                # scores S[t,h,g] = sum_d q[t,h,d] k[t,g,d] (q pre-scaled)
                tok = toks[tt][0]
                S = att.tile([128, H, H], F32, tag="S")
                ks = tok[:, D : 2 * D]
                prod = prodp.tile([128, H, H, HD], ATT_DT, tag="prod")
                qs = tok[:, 0 : H * HD]
                nc.vector.tensor_tensor(
                    out=prod,
                    in0=_ap(qs, [[HD, H], [0, H], [1, HD]]),
                    in1=_ap(ks, [[0, H], [HD, H], [1, HD]]),
                    op=ALU.mult,
                )
                sdst = _ap(S[:, 0, 0], [[H, H], [1, H], [1, 1]])
                halving_tree(prod, H, H, HD, sdst)
                Ss.append(S)

            for tt in range(NT):
                vt = toks[tt][1]
                # softmax over g without max-subtraction (|S| is O(5));
                # all side-ops on DVE (Pool contends with DVE for SBUF ports)
                Sx = att.tile([128, H, H], ATT_DT, tag="Sx")
                nc.scalar.activation(out=Sx, in_=Ss[tt], func=Exp)
                stats = att.tile([128, 2, H], F32, tag="stats")
                nc.vector.tensor_reduce(
                    out=stats[:, 0, :], in_=Sx, axis=AX.X, op=ALU.add
                )
                nc.vector.reciprocal(stats[:, 1, :], stats[:, 0, :])
                P = att.tile([128, H, H], ATT_DT, tag="P")
                nc.vector.tensor_tensor(
                    out=P, in0=Sx, in1=_ap(stats[:, 1, 0], [[1, H], [0, H]]),
                    op=ALU.mult,
                )

                # o[t,h,d] = sum_g P[t,h,g] v[t,g,d]
                O = outp.tile([128, H, HD], ATT_DT, tag="O")
                prod2 = prodp.tile([128, H, HD, H], ATT_DT, tag="prod")
                nc.vector.tensor_tensor(
                    out=prod2,
                    in0=_ap(P[:, 0, :], [[H, H], [0, HD], [1, H]]),
                    in1=_ap(vt[:, :], [[0, H], [H, HD], [1, H]]),
                    op=ALU.mult,
                )
                odst = _ap(O[:, 0, :], [[HD, H], [1, HD], [1, 1]])
                halving_tree(prod2, H, HD, H, odst)

                # spill head-major: dst[h, t, d] = O[t, h, d]
                base = aspill[n]
                for h0 in range(0, H, H // 2):
                    dst = bass.AP(
                        tensor=base.tensor,
                        offset=base.offset + h0 * T * HD + (t0 + tt * 128) * HD,
                        ap=[[HD, 128], [T * HD, H // 2], [1, HD]],
                    )
                    nc.sync.dma_start(out=dst, in_=O[:, h0 : h0 + H // 2, :])

        def p2_chunk(n, c, tail=False):
            t0 = c * CH
            AT = xtp.tile([128, KD, CH], ATT_DT, tag="AT")
            for tt in range(NT):
                at = xp.tile([128, D], ATT_DT, tag="a")
                nc.scalar.dma_start(
                    out=at, in_=aspill[n, t0 + tt * 128 : t0 + tt * 128 + 128, :]
                )
                for k in range(0, KD, 2):
                    pt = psT.tile([128, 2, 128], ATT_DT, tag="tp")
                    for kk in range(2):
                        nc.tensor.transpose(
                            pt[:, kk, :],
                            at[:, (k + kk) * 128 : (k + kk + 1) * 128],
                            ident_b,
                        )
                    cp = nc.any.tensor_copy if tail else nc.scalar.copy
                    cp(
                        out=_ap(AT[:, k, tt * 128], [[CH, 2], [1, 128]]),
                        in_=pt,
                    )
            yT = ytp.tile([128, JP, CH], ATT_DT, tag="yT")
            for j in range(JP):
                pm = psA.tile([128, CH], F32, tag="mm")
                for k in range(KD):
                    nc.tensor.matmul(
                        pm,
                        wp_s[:, k, j * 128 : (j + 1) * 128],
                        AT[:, k, :],
                        start=(k == 0),
                        stop=(k == KD - 1),
                    )
                nc.scalar.activation(
                    out=yT[:, j, :], in_=pm, func=Ident,
                    bias=bias[:, JQ + j : JQ + j + 1], scale=1.0,
                )
            for tt in range(NT):
                yt = ytp.tile([128, D], F32, tag="y")
                for j in range(0, JP, 2):
                    pt = psT.tile([128, 2, 128], ATT_DT, tag="tp")
                    for jj in range(2):
                        nc.tensor.transpose(
                            pt[:, jj, :],
                            yT[:, j + jj, tt * 128 : (tt + 1) * 128],
                            ident_b,
                        )
                    cp = nc.any.tensor_copy if tail else nc.scalar.copy
                    cp(out=yt[:, j * 128 : (j + 2) * 128], in_=pt)
                r0 = n * T + t0 + tt * 128
                nc.sync.dma_start(out=y[r0 : r0 + 128, :], in_=yt)

        # interleave phase-2 (PE/ACT-heavy) one batch element behind phase-1
        # (DVE-heavy attention): p2(n, c) projects head-c rows spanning ALL
        # tokens of batch element n (the reference's h-major flatten), so it
        # can only start once every p1(n, *) chunk has spilled
        for c in range(NCH):
            p1_chunk(0, c)
        for n in range(1, NB):
            for c in range(NCH):
                p1_chunk(n, c)
                p2_chunk(n - 1, c)
        for c in range(NCH):
            p2_chunk(NB - 1, c, tail=True)

    # TRN2 allows at most one sync wait per engine instruction; split
    # multi-wait instructions through event semaphores.
    import bass_rust

    bass_rust.generate_event_semaphores(nc)
    return nc


_NC_CACHE = None
TRACE = False
LAST_RESULTS = None


def kernel(x, w_qkv, b_qkv, w_proj, b_proj):
    global _NC_CACHE, LAST_RESULTS
    if _NC_CACHE is None:
        _NC_CACHE = build_kernel()
    nc = _NC_CACHE
    x = np.ascontiguousarray(np.asarray(x, dtype=np.float32))
    w_qkv = np.ascontiguousarray(np.asarray(w_qkv, dtype=np.float32))
    b_qkv = np.ascontiguousarray(np.asarray(b_qkv, dtype=np.float32))
    w_proj = np.ascontiguousarray(np.asarray(w_proj, dtype=np.float32))
    b_proj = np.ascontiguousarray(np.asarray(b_proj, dtype=np.float32))
    in_maps = []
    for i in range(NCORES):
        in_maps.append(
            {
                "x": x[i * NB : (i + 1) * NB].reshape(NB * T, D),
                "w_qkv": w_qkv,
                "b_qkv": b_qkv,
                "w_proj": w_proj,
                "b_proj": b_proj,
            }
        )
    res = run_bass_kernel_spmd(
        nc, in_maps, core_ids=list(range(NCORES)), trace=TRACE
    )
    LAST_RESULTS = res
    out = np.empty((N, T, D), dtype=np.float32)
    for i in range(NCORES):
        out[i * NB : (i + 1) * NB] = res.results[i]["y"].reshape(NB, T, D)
    return out


if __name__ == "__main__":
    rng = np.random.default_rng(0)
    inputs = {
        "x": rng.standard_normal((N, T, D), dtype=np.float32),
        "w_qkv": rng.standard_normal((D, 3 * D), dtype=np.float32) * D**-0.5,
        "b_qkv": rng.standard_normal((3 * D,), dtype=np.float32) * 0.02,
        "w_proj": rng.standard_normal((D, D), dtype=np.float32) * D**-0.5,
        "b_proj": rng.standard_normal((D,), dtype=np.float32) * 0.02,
    }
    out = kernel(**inputs)
    print("out", out.shape, out.dtype)



# revision 28
# speedup vs baseline: 3.7359x; 1.6352x over previous
"""Trainium2 Bass kernel for per-token head-attention transformer block.

Reference computation (N=16, T=4096, D=1024, H=16, hd=64):
    qkv = x @ w_qkv + b_qkv                       (N,T,3D)
    q,k,v = split(qkv)  each (N,T,H,hd)
    S = einsum('nthd,ntgd->nthg', q*hd^-0.5, k)   per-token 16x16 over heads
    P = softmax(S, -1)
    o = einsum('nthg,ntgd->nthd', P, v)
    out = o.transpose(0,2,1,3).reshape(N,T,D) @ w_proj + b_proj

Mapping: data-parallel over batch N across 8 cores (2 batch elements each).
Per core the kernel is DVE-bound: the per-token 16x16 attention costs ~32K
MACs/token which runs at ~1 MAC/cycle/lane as bf16 tensor_tensor product +
halving add-tree (~40us per 128-token tile on HW incl. ~0.4us/op overhead).
Design points (HW-measured rationale in the session notes):
  - ALL attention math on DVE. GPSIMD shares its SBUF port with DVE;
    measured: independent Pool tensor_tensor overlaps only ~15% with a busy
    DVE stream, so any head-group offload to Pool is a net loss.
  - per-chunk emission order: tok/v marshalling for BOTH tiles, then QK for
    both tiles back-to-back on DVE, then per-tile softmax+AV. The tile-tt
    exp (ACT) lands during tile-(tt+1)'s QK, so softmax never stalls DVE,
    and ACT's queue never has an exp ahead of evacuation work
    (head-of-line blocking starves PE via psT/psA backpressure otherwise).
  - all matmuls in bf16 (same PE rate as f32r, half the SBUF, FWL weight loads)
  - one fused product per tile+phase, then an in-place halving add-tree;
    the last level writes S / O directly
  - no max-subtraction in softmax (scores are O(5) for this data; exp in f32)
  - v is written straight into its (d,g)-interleaved tile during the
    PSUM-evacuation of the v transposes
  - PSUM evacuations all on ACT; DVE does only attention math + softmax
    side-ops (reduce/recip/normalize are cheaper on DVE than the cross-engine
    stall they cause elsewhere)
  - attention output spilled to DRAM in bf16; p2(n, c) projects head-c rows
    of the reference's h-major (T,D) view, which span ALL tokens of batch
    element n, so p2 lags p1 by one full batch element
Measured on HW: baseline 4.19ms -> 2.92ms. Failed experiments (reverted):
head-splits to Pool (port contention), tile-merged tree ops (exp head-of-line
cascade, 3.44ms), per-head spill DMAs + half-granular p2 (SWDGE DMA storm,
7.6ms), reciprocal_approx_fast (neuronxcc "ISA wrong length").
"""

import sys

sys.path.insert(0, "/opt/trn_rl_repo")

from contextlib import ExitStack

import numpy as np

import concourse.bass as bass
import concourse.tile as tile
from concourse import mybir
from concourse.bass_utils import run_bass_kernel_spmd
from concourse.masks import make_identity

N, T, D = 16, 4096, 1024
H, HD = 16, 64
NCORES = 8
NB = N // NCORES  # batch elements per core
SCALE = float(HD) ** -0.5

F32 = mybir.dt.float32
BF16 = mybir.dt.bfloat16
ATT_DT = BF16

CH = 256          # token chunk (matmul moving dim)
NT = CH // 128    # token tiles per chunk
KD = D // 128     # contraction chunks (8)
JQ = 3 * D // 128  # qkv output feature chunks (24)
JP = D // 128     # proj output feature chunks (8)
NCH = T // CH     # chunks per batch element (16)

# head-group split across the two vector engines: list of (h0, nh, engine)
# engine: 'v' = DVE, 'p' = GPSIMD
# GPSIMD (Pool) shares its SBUF port with DVE: HW-measured, independent Pool
# tensor_tensor ops overlap only ~15% with a busy DVE stream, so offloading
# head-groups to Pool is a net loss. All attention math runs on DVE.
QK_GROUPS = [(0, 16, "v")]
AV_GROUPS = [(0, 16, "v")]

Ident = mybir.ActivationFunctionType.Identity
Exp = mybir.ActivationFunctionType.Exp
ALU = mybir.AluOpType
AX = mybir.AxisListType


def _ap(sl, dims):
    """Custom free-dim access pattern on a sliced tile: keep partition dim +
    offset of `sl`, replace free dims with [step, num] list `dims`."""
    return bass.AP(tensor=sl.tensor, offset=sl.offset, ap=[sl.ap[0]] + dims)


def build_kernel():
    nc = bass.Bass()
    x = nc.dram_tensor("x", [NB * T, D], F32, kind="ExternalInput")
    wqkv = nc.dram_tensor("w_qkv", [D, 3 * D], F32, kind="ExternalInput")
    bqkv = nc.dram_tensor("b_qkv", [3 * D], F32, kind="ExternalInput")
    wproj = nc.dram_tensor("w_proj", [D, D], F32, kind="ExternalInput")
    bproj = nc.dram_tensor("b_proj", [D], F32, kind="ExternalInput")
    y = nc.dram_tensor("y", [NB * T, D], F32, kind="ExternalOutput")

    with ExitStack() as ctx:
        tc = ctx.enter_context(tile.TileContext(nc))
        singles = ctx.enter_context(tc.tile_pool(name="singles", bufs=1))
        xp = ctx.enter_context(tc.tile_pool(name="xp", bufs=2))
        xtp = ctx.enter_context(tc.tile_pool(name="xtp", bufs=3))
        ytp = ctx.enter_context(tc.tile_pool(name="ytp", bufs=1))
        qkvp = ctx.enter_context(tc.tile_pool(name="qkvp", bufs=1))
        tokp = ctx.enter_context(tc.tile_pool(name="tokp", bufs=2))
        att = ctx.enter_context(tc.tile_pool(name="att", bufs=2))
        outp = ctx.enter_context(tc.tile_pool(name="outp", bufs=2))
        prodp = ctx.enter_context(tc.tile_pool(name="prodp", bufs=1))
        vtp = ctx.enter_context(tc.tile_pool(name="vtp", bufs=2))
        psA = ctx.enter_context(tc.tile_pool(name="psA", bufs=3, space="PSUM"))
        psT = ctx.enter_context(tc.tile_pool(name="psT", bufs=5, space="PSUM"))
        dram = ctx.enter_context(tc.tile_pool(name="dram", bufs=1, space="DRAM"))

        ident = singles.tile([128, 128], F32)
        make_identity(nc, ident)
        ident_b = singles.tile([128, 128], ATT_DT)
        make_identity(nc, ident_b)

        # resident weights in bf16, (in,out) layout chunked over contraction dim
        wq_s = singles.tile([128, KD, 3 * D], BF16)
        wq_src = wqkv.rearrange("(k p) j -> p k j", p=128)
        for k in range(KD):
            nc.gpsimd.dma_start(out=wq_s[:, k, :], in_=wq_src[:, k, :])
        wp_s = singles.tile([128, KD, D], BF16)
        wp_src = wproj.rearrange("(k p) j -> p k j", p=128)
        for k in range(KD):
            nc.gpsimd.dma_start(out=wp_s[:, k, :], in_=wp_src[:, k, :])
        # biases, one merged tile: cols [0,JQ) = b_qkv, [JQ,JQ+JP) = b_proj,
        # [JQ+JP, ...) = SCALE * b_q
        bias = singles.tile([128, JQ + JP + JQ // 3], F32)
        nc.gpsimd.dma_start(
            out=bias[:, 0:JQ], in_=bqkv.rearrange("(j p) -> p j", p=128)
        )
        nc.gpsimd.dma_start(
            out=bias[:, JQ : JQ + JP], in_=bproj.rearrange("(j p) -> p j", p=128)
        )
        nc.scalar.mul(bias[:, JQ + JP :], bias[:, 0 : JQ // 3], SCALE)

        # head-major attention-output spill (bf16): flat layout
        # h*(T*HD) + t*HD + d, viewed by phase 2 as a row-major (T, D)
        # matrix per batch element.
        aspill = dram.tile([NB, T, D], ATT_DT)

        def halving_tree(prod, nh, mid, inner, out_final):
            """Sum the innermost dim of prod [128, nh, mid, inner] by
            repeated in-place halving adds (out aliases the low half of
            the input range); the final level writes out_final."""
            m = inner
            while m > 1:
                half = m // 2
                if half == 1:
                    dst = out_final
                else:
                    dst = prod[:, :, :, 0:half]
                nc.vector.tensor_tensor(
                    out=dst,
                    in0=prod[:, :, :, 0:half],
                    in1=prod[:, :, :, half:m],
                    op=ALU.add,
                )
                m = half

        def p1_chunk(n, c):
            t0 = c * CH
            xT = xtp.tile([128, KD, CH], BF16, tag="xT")
            for tt in range(NT):
                xt = xp.tile([128, D], F32, tag="x")
                r0 = n * T + t0 + tt * 128
                nc.sync.dma_start(out=xt, in_=x[r0 : r0 + 128, :])
                for k in range(0, KD, 2):
                    pt = psT.tile([128, 2, 128], F32, tag="tp")
                    for kk in range(2):
                        nc.tensor.transpose(
                            pt[:, kk, :],
                            xt[:, (k + kk) * 128 : (k + kk + 1) * 128],
                            ident,
                        )
                    nc.scalar.copy(
                        out=_ap(xT[:, k, tt * 128], [[CH, 2], [1, 128]]),
                        in_=pt,
                    )

            qkvT = qkvp.tile([128, JQ, CH], ATT_DT, tag="qkvT")
            for j in range(JQ):
                pm = psA.tile([128, CH], F32, tag="mm")
                for k in range(KD):
                    nc.tensor.matmul(
                        pm,
                        wq_s[:, k, j * 128 : (j + 1) * 128],
                        xT[:, k, :],
                        start=(k == 0),
                        stop=(k == KD - 1),
                    )
                if j < JQ // 3:  # q: fold in attention scale
                    nc.scalar.activation(
                        out=qkvT[:, j, :], in_=pm, func=Ident,
                        bias=bias[:, JQ + JP + j : JQ + JP + j + 1], scale=SCALE,
                    )
                else:
                    nc.scalar.activation(
                        out=qkvT[:, j, :], in_=pm, func=Ident,
                        bias=bias[:, j : j + 1], scale=1.0,
                    )

            # token-major marshalling for BOTH tiles first, so the ACT queue
            # never has an exp (which waits on DVE) ahead of evac work
            toks = []
            for tt in range(NT):
                # feature-major -> token-major for the per-token attention;
                # q,k go to `tok`, v goes straight into the (d,g) layout
                tok = tokp.tile([128, 2 * D], ATT_DT, tag="tok")
                vt = vtp.tile([128, HD, H], ATT_DT, tag="vt")
                for j in range(0, JQ, 2):
                    pt = psT.tile([128, 2, 128], ATT_DT, tag="tp")
                    for jj in range(2):
                        nc.tensor.transpose(
                            pt[:, jj, :],
                            qkvT[:, j + jj, tt * 128 : (tt + 1) * 128],
                            ident_b,
                        )
                    if j < 2 * JQ // 3:
                        nc.scalar.copy(
                            out=tok[:, j * 128 : (j + 2) * 128], in_=pt
                        )
                    else:
                        jv = j - 2 * JQ // 3
                        nc.scalar.copy(
                            out=_ap(
                                vt[:, 0, 2 * jv], [[2, 2], [1, 2], [H, HD]]
                            ),
                            in_=_ap(
                                pt[:, 0, 0], [[128, 2], [HD, 2], [1, HD]]
                            ),
                        )
                toks.append((tok, vt))

            # QK for both tiles back-to-back on DVE; the tile-tt exp (ACT)
            # then lands during tile-(tt+1)'s QK, so the later softmax ops
            # never stall the DVE stream
            Ss = []
            for tt in range(NT):
                # scores S[t,h,g] = sum_d q[t,h,d] k[t,g,d] (q pre-scaled)
                tok = toks[tt][0]
                S = att.tile([128, H, H], F32, tag="S")
                ks = tok[:, D : 2 * D]
                prod = prodp.tile([128, H, H, HD], ATT_DT, tag="qkv")
                qs = tok[:, 0 : H * HD]
                nc.vector.tensor_tensor(
                    out=prod,
                    in0=_ap(qs, [[HD, H], [0, H], [1, HD]]),
                    in1=_ap(ks, [[0, H], [HD, H], [1, HD]]),
                    op=ALU.mult,
                )
                sdst = _ap(S[:, 0, 0], [[H, H], [1, H], [1, 1]])
                halving_tree(prod, H, H, HD, sdst)
                Ss.append(S)

            for tt in range(NT):
                vt = toks[tt][1]
                # softmax over g without max-subtraction (|S| is O(5));
                # all side-ops on DVE (Pool contends with DVE for SBUF ports)
                Sx = att.tile([128, H, H], ATT_DT, tag="Sx")
                nc.scalar.activation(out=Sx, in_=Ss[tt], func=Exp)
                stats = att.tile([128, 2, H], F32, tag="stats")
                nc.vector.tensor_reduce(
                    out=stats[:, 0, :], in_=Sx, axis=AX.X, op=ALU.add
                )
                nc.vector.reciprocal(stats[:, 1, :], stats[:, 0, :])
                P = att.tile([128, H, H], ATT_DT, tag="P")
                nc.vector.tensor_tensor(
                    out=P, in0=Sx, in1=_ap(stats[:, 1, 0], [[1, H], [0, H]]),
                    op=ALU.mult,
                )

                # o[t,h,d] = sum_g P[t,h,g] v[t,g,d]
                O = outp.tile([128, H, HD], ATT_DT, tag="O")
                prod2 = prodp.tile([128, H, HD, H], ATT_DT, tag="avv")
                nc.vector.tensor_tensor(
                    out=prod2,
                    in0=_ap(P[:, 0, :], [[H, H], [0, HD], [1, H]]),
                    in1=_ap(vt[:, :], [[0, H], [H, HD], [1, H]]),
                    op=ALU.mult,
                )
                odst = _ap(O[:, 0, :], [[HD, H], [1, HD], [1, 1]])
                halving_tree(prod2, H, HD, H, odst)

                # spill head-major: dst[h, t, d] = O[t, h, d]
                base = aspill[n]
                for h0 in range(0, H, H // 2):
                    dst = bass.AP(
                        tensor=base.tensor,
                        offset=base.offset + h0 * T * HD + (t0 + tt * 128) * HD,
                        ap=[[HD, 128], [T * HD, H // 2], [1, HD]],
                    )
                    nc.sync.dma_start(out=dst, in_=O[:, h0 : h0 + H // 2, :])

        def p2_chunk(n, c, tail=False):
            t0 = c * CH
            AT = xtp.tile([128, KD, CH], ATT_DT, tag="AT")
            for tt in range(NT):
                at = xp.tile([128, D], ATT_DT, tag="a")
                nc.scalar.dma_start(
                    out=at, in_=aspill[n, t0 + tt * 128 : t0 + tt * 128 + 128, :]
                )
                for k in range(0, KD, 2):
                    pt = psT.tile([128, 2, 128], ATT_DT, tag="tp")
                    for kk in range(2):
                        nc.tensor.transpose(
                            pt[:, kk, :],
                            at[:, (k + kk) * 128 : (k + kk + 1) * 128],
                            ident_b,
                        )
                    cp = nc.any.tensor_copy if tail else nc.scalar.copy
                    cp(
                        out=_ap(AT[:, k, tt * 128], [[CH, 2], [1, 128]]),
                        in_=pt,
                    )
            yT = ytp.tile([128, JP, CH], ATT_DT, tag="yT")
            for j in range(JP):
                pm = psA.tile([128, CH], F32, tag="mm")
                for k in range(KD):
                    nc.tensor.matmul(
                        pm,
                        wp_s[:, k, j * 128 : (j + 1) * 128],
                        AT[:, k, :],
                        start=(k == 0),
                        stop=(k == KD - 1),
                    )
                nc.scalar.activation(
                    out=yT[:, j, :], in_=pm, func=Ident,
                    bias=bias[:, JQ + j : JQ + j + 1], scale=1.0,
                )
            for tt in range(NT):
                yt = ytp.tile([128, D], F32, tag="y")
                for j in range(0, JP, 2):
                    pt = psT.tile([128, 2, 128], ATT_DT, tag="tp")
                    for jj in range(2):
                        nc.tensor.transpose(
                            pt[:, jj, :],
                            yT[:, j + jj, tt * 128 : (tt + 1) * 128],
                            ident_b,
                        )
                    cp = nc.any.tensor_copy if tail else nc.scalar.copy
                    cp(out=yt[:, j * 128 : (j + 2) * 128], in_=pt)
                r0 = n * T + t0 + tt * 128
                nc.sync.dma_start(out=y[r0 : r0 + 128, :], in_=yt)

        # interleave phase-2 (PE/ACT-heavy) one batch element behind phase-1
        # (DVE-heavy attention): p2(n, c) projects head-c rows spanning ALL
        # tokens of batch element n (the reference's h-major flatten), so it
        # can only start once every p1(n, *) chunk has spilled
        for c in range(NCH):
            p1_chunk(0, c)
        for n in range(1, NB):
            for c in range(NCH):
                p1_chunk(n, c)
                p2_chunk(n - 1, c)
        for c in range(NCH):
            p2_chunk(NB - 1, c, tail=True)

    # TRN2 allows at most one sync wait per engine instruction; split
    # multi-wait instructions through event semaphores.
    import bass_rust

    bass_rust.generate_event_semaphores(nc)
    return nc


_NC_CACHE = None
TRACE = False
LAST_RESULTS = None


def kernel(x, w_qkv, b_qkv, w_proj, b_proj):
    global _NC_CACHE, LAST_RESULTS
    if _NC_CACHE is None:
        _NC_CACHE = build_kernel()
    nc = _NC_CACHE
    x = np.ascontiguousarray(np.asarray(x, dtype=np.float32))
    w_qkv = np.ascontiguousarray(np.asarray(w_qkv, dtype=np.float32))
    b_qkv = np.ascontiguousarray(np.asarray(b_qkv, dtype=np.float32))
    w_proj = np.ascontiguousarray(np.asarray(w_proj, dtype=np.float32))
    b_proj = np.ascontiguousarray(np.asarray(b_proj, dtype=np.float32))
    in_maps = []
    for i in range(NCORES):
        in_maps.append(
            {
                "x": x[i * NB : (i + 1) * NB].reshape(NB * T, D),
                "w_qkv": w_qkv,
                "b_qkv": b_qkv,
                "w_proj": w_proj,
                "b_proj": b_proj,
            }
        )
    res = run_bass_kernel_spmd(
        nc, in_maps, core_ids=list(range(NCORES)), trace=TRACE
    )
    LAST_RESULTS = res
    out = np.empty((N, T, D), dtype=np.float32)
    for i in range(NCORES):
        out[i * NB : (i + 1) * NB] = res.results[i]["y"].reshape(NB, T, D)
    return out


if __name__ == "__main__":
    rng = np.random.default_rng(0)
    inputs = {
        "x": rng.standard_normal((N, T, D), dtype=np.float32),
        "w_qkv": rng.standard_normal((D, 3 * D), dtype=np.float32) * D**-0.5,
        "b_qkv": rng.standard_normal((3 * D,), dtype=np.float32) * 0.02,
        "w_proj": rng.standard_normal((D, D), dtype=np.float32) * D**-0.5,
        "b_proj": rng.standard_normal((D,), dtype=np.float32) * 0.02,
    }
    out = kernel(**inputs)
    print("out", out.shape, out.dtype)

